# revision 7
# baseline (speedup 1.0000x reference)
"""Trainium2 Bass kernel for multi-head attention (B=2, P=2048, M=1024, N=16, H=64).

out = softmax(mask(x@Wq @ (x@Wk)^T / sqrt(H))) @ (x@Wv) @ Wo + biases,
with the module's strictly-upper-triangular keep mask (row P-1 fully masked).

Sharding: 8 cores = 2 batches x 4 head-groups. Core c handles batch c//4,
heads [4*(c%4), 4*(c%4)+4). Each core computes its heads' attention and the
partial output projection; the host sums partials across the 4 cores of each
batch.

Device algorithm (per core; bf16 matmuls, fp32 PSUM accumulation):
  - All projection weights arrive as ONE contiguous [128, 6*1024] DMA (host
    pre-packs the m-chunk-major layout); x^T arrives as 8 linear 512KB chunk
    DMAs alternating between the two HWDGE rings so the first projection
    matmul starts ~2us in.
  - q^T,k^T [h', p] via projections with x^T as the moving operand; QKV
    biases are folded into the PSUM evacuation (ACT Identity + per-partition
    bias AP), so no K=1 bias matmuls.
  - v^T slices + an appended ones row are PE-transposed (bf16) into merged
    [128, 4*65] PSUM tiles -> one DVE evacuation per 4 chunks. Column 64 of
    each 65-block is ones, so the z matmul also produces softmax
    denominators.
  - scores^T [pk, pq] with the triangular mask exploited by skipping
    fully-masked pk-chunks and narrowing partial ones. The two heads of a
    pair run CONCURRENTLY in disjoint PE row-groups (K=64 each), one ScalarE
    exp covers both heads; mask applied multiplicatively after exp.
  - z matmuls trail the scores/exp stream by a sliding window of DW slots.
  - Normalization WITHOUT transposes: the two denominator rows are
    reciprocal'd ([1,512] DVE ops from PSUM), broadcast down 128 partitions
    by one K=2 fp32 matmul against a constant selector, and multiplied into
    z^T by two DVE tensor_muls -> zp [128 (2 heads x 64), 512 pq] bf16,
    which is directly the lhsT for the output projection.
  - The fully-masked query row P-1 is patched analytically on the host:
    out[b,P-1] = sum_n (mean_p x[b] @ Wv[n] + bv[n]) @ Wo[n] + bias_out.
"""
import sys

import numpy as np

if "/opt/trn_rl_repo" not in sys.path:
    sys.path.insert(0, "/opt/trn_rl_repo")

import concourse.bacc as bacc
import concourse.tile as tile
from concourse import mybir
from concourse import bass_utils
import ml_dtypes

B, P, M, N, H = 2, 2048, 1024, 16, 64
NCORES = 8
HPC = 4          # heads per core
NPAIRS = 2       # head pairs per core
MK = M // 128    # 8 contraction chunks for projections
PT = P // 512    # 4 free-dim tiles of 512 over sequence
PC = P // 128    # 16 partition chunks over sequence
MT = M // 512    # 2 output m-tiles

F32 = mybir.dt.float32
BF16 = mybir.dt.bfloat16
EXP = mybir.ActivationFunctionType.Exp

_BUILT = {}


def _emit(nc, tc, aps, ctx):
    xT = aps["xT"]          # [1024, 2048]
    outp = aps["outp"]      # [2048, 1024]

    consts = ctx.enter_context(tc.tile_pool(name="consts", bufs=1))
    xpool = ctx.enter_context(tc.tile_pool(name="xpool", bufs=MK))
    qkpool = ctx.enter_context(tc.tile_pool(name="qkpool", bufs=4))
    vapool = ctx.enter_context(tc.tile_pool(name="vapool", bufs=18))
    zppool = ctx.enter_context(tc.tile_pool(name="zppool", bufs=9))
    expool = ctx.enter_context(tc.tile_pool(name="expool", bufs=9))
    rcpool = ctx.enter_context(tc.tile_pool(name="rcpool", bufs=2))
    bcpool = ctx.enter_context(tc.tile_pool(name="bcpool", bufs=2))
    opool = ctx.enter_context(tc.tile_pool(name="opool", bufs=4))
    vtpool = ctx.enter_context(tc.tile_pool(name="vtpool", bufs=4))

    # ---- constants + weights, packed for contiguous DMA ----
    bb = consts.tile([128, 6], F32, tag="bb")       # qkv biases, both pairs
    nc.sync.dma_start(bb[:], aps["bb"][:])
    eye = consts.tile([66, 66], BF16, tag="eye")
    nc.sync.dma_start(eye[:], aps["eye"][:])
    mask = consts.tile([128, 128], BF16, tag="mask")
    nc.sync.dma_start(mask[:], aps["mask"][:])
    ones1 = consts.tile([1, 128], F32, tag="ones1")
    nc.vector.memset(ones1[:], 1.0)

    # wall: [wv0 | wq0 | wk0 | wv1 | wq1 | wk1], each [128, MK*128]
    wall = consts.tile([128, 6 * 1024], BF16, tag="wall")
    nc.scalar.dma_start(wall[:, 0:1024], aps["wall"][:, 0:1024])
    nc.scalar.dma_start(wall[:, 1024:6144], aps["wall"][:, 1024:6144])
    wot = consts.tile([128, 2048], BF16, tag="wo")
    nc.scalar.dma_start(wot[:], aps["wo"][:])

    def wslice(t, pr, mk):
        base = 1024 * (3 * pr + {"v": 0, "q": 1, "k": 2}[t])
        return wall[:, base + 128 * mk:base + 128 * (mk + 1)]

    def bslice(t, pr):
        col = 3 * pr + {"v": 0, "q": 1, "k": 2}[t]
        return bb[:, col:col + 1]

    # x^T chunks [128 m, 2048 p], linear 512KB each, alternating rings
    xsb = []
    for k in range(MK):
        xt = xpool.tile([128, 2048], BF16, tag="x")
        eng = nc.sync if k % 2 == 0 else nc.scalar
        eng.dma_start(xt[:], xT[128 * k:128 * (k + 1), :])
        xsb.append(xt)

    tiles = {}
    qts, kts = {}, {}
    vts_set = []

    def finish_pair(pr, j, zpss, bc_pool):
        """Normalize both heads' z^T without transposes: reciprocal the
        denominator rows, broadcast down partitions via one K=2 matmul,
        multiply into z^T -> zp [128, 512] bf16 (lhsT of the out proj)."""
        if j == PT - 1:
            # fully-masked query row P-1: denom 0 -> 1 so the reciprocal
            # is finite (host patches the output row)
            nc.vector.memset(zpss[0][64:65, 511:512], 1.0)
            nc.vector.memset(zpss[1][64:65, 511:512], 1.0)
        rcs = [rcpool.tile([1, 512], F32, tag=f"rc{h01}",
                           name=f"rc{pr}_{j}_{h01}")
               for h01 in range(2)]
        nc.vector.reciprocal(rcs[0][:], zpss[0][64:65, :])
        nc.vector.reciprocal(rcs[1][:], zpss[1][64:65, :])
        bcps = bc_pool.tile([128, 512], F32, tag="tps", name=f"bc{pr}_{j}")
        for h01 in range(2):
            nc.tensor.matmul(bcps[64 * h01:64 * (h01 + 1), :],
                             ones1[:, 0:64], rcs[h01][:],
                             start=True, stop=True)
        bcs = bcpool.tile([128, 512], F32, tag="bcs")
        nc.scalar.copy(bcs[:], bcps[:])
        zp = zppool.tile([128, 512], BF16, tag="zp", name=f"zp{pr}_{j}")
        nc.vector.tensor_mul(zp[0:64, :], zpss[0][0:64, :], bcs[0:64, :])
        nc.vector.tensor_mul(zp[64:128, :], zpss[1][0:64, :], bcs[64:128, :])
        tiles[("zp", pr, j)] = zp

    def proj(j, ps_pool):
        for c4 in range(4):
            ck = 4 * j + c4
            for mt in range(MT):
                pp = ps_pool.tile([128, 512], F32, tag="tps", bufs=2,
                                  name=f"prps{ck}_{mt}")
                nc.tensor.matmul(
                    pp[:], tiles[("zp", 0, j)][:, 128 * c4:128 * (c4 + 1)],
                    wot[:, 1024 * 0 + 512 * mt:1024 * 0 + 512 * (mt + 1)],
                    start=True, stop=False,
                )
                nc.tensor.matmul(
                    pp[:], tiles[("zp", 1, j)][:, 128 * c4:128 * (c4 + 1)],
                    wot[:, 1024 * 1 + 512 * mt:1024 * 1 + 512 * (mt + 1)],
                    start=False, stop=True,
                )
                osb = opool.tile([128, 512], F32, tag="osb")
                if (c4 + mt) % 2 == 0:
                    nc.vector.tensor_copy(osb[:], pp[:])
                else:
                    nc.scalar.copy(osb[:], pp[:])
                nc.sync.dma_start(
                    outp[128 * ck:128 * (ck + 1), 512 * mt:512 * (mt + 1)],
                    osb[:],
                )

    def va_slice(pr, h01, i_):
        t, c4 = tiles[("va", pr, h01, i_ // 4)], i_ % 4
        return t[:, 66 * c4:66 * c4 + 65]

    def attn(pr, j, sc_pool, z_pool, bc_pool):
        """Row-packed attention: both heads' K=64 score matmuls run
        concurrently in disjoint PE row-groups into one [128,1024] PSUM
        tile; one batched exp covers both. z matmuls trail by DW slots."""
        qT, kT = qts[pr], kts[pr]
        ilist = list(range(PC - 1, 4 * j - 1, -1))
        nslot = len(ilist)
        DW = min(6, nslot - 1)
        zpss = [z_pool.tile([65, 512], F32, tag="zps", bufs=2,
                            name=f"zps{pr}_{h01}_{j}")
                for h01 in range(2)]
        descs = []
        for idx in range(nslot + DW):
            if idx < nslot:
                i_ = ilist[idx]
                tt = i_ - 4 * j
                w_ = min(512, 128 * (tt + 1))
                sp = sc_pool.tile([128, 1024], F32, tag="scps", bufs=2,
                                  name=f"sps{pr}_{j}_{i_}")
                halves = [sp[:, :w_], sp[:, 512:512 + w_]]
                for h01 in range(2):
                    rows = slice(64 * h01, 64 * (h01 + 1))
                    nc.tensor.matmul(
                        halves[h01],
                        kT[rows, 128 * i_:128 * (i_ + 1)],
                        qT[rows, 512 * j:512 * j + w_],
                        start=True, stop=True,
                    )
                ex = expool.tile([128, 1024], BF16, tag="ex")
                if w_ == 512:
                    nc.scalar.activation(ex[:], sp[:], EXP, scale=0.125)
                else:
                    nc.scalar.activation(ex[:, :w_], halves[0], EXP,
                                         scale=0.125)
                    nc.scalar.activation(ex[:, 512:512 + w_], halves[1],
                                         EXP, scale=0.125)
                if tt < 4:
                    for h01 in range(2):
                        off = 512 * h01
                        nc.vector.tensor_mul(
                            ex[:, off + 128 * tt:off + w_],
                            ex[:, off + 128 * tt:off + w_], mask[:]
                        )
                descs.append((ex, i_, w_))
            zi = idx - DW
            if 0 <= zi < nslot:
                ex, i_, w_ = descs[zi]
                for h01 in range(2):
                    nc.tensor.matmul(
                        zpss[h01][:, :w_], va_slice(pr, h01, i_),
                        ex[:, 512 * h01:512 * h01 + w_],
                        start=(zi == 0), stop=(zi == nslot - 1),
                    )
        finish_pair(pr, j, zpss, bc_pool)

    # ---- QKV projections ----
    with tc.tile_pool(name="ps_qkv", bufs=3, space="PSUM") as ps_qkv, \
         tc.tile_pool(name="ps_vt", bufs=1, space="PSUM") as ps_vt:
        for pr in range(NPAIRS):
            qT = qkpool.tile([128, 2048], BF16, tag="qT", name=f"qT{pr}")
            kT = qkpool.tile([128, 2048], BF16, tag="kT", name=f"kT{pr}")
            qts[pr], kts[pr] = qT, kT
            # v first: va tiles feed the z matmuls of the j=3 attention
            for j4a in range(0, PT, 2):
                pss = [ps_qkv.tile([128, 512], F32, tag="qkvps",
                                   name=f"ps_v{pr}{j4a + d}")
                       for d in range(2)]
                for mk in range(MK):
                    for d in range(2):
                        nc.tensor.matmul(
                            pss[d][:], wslice("v", pr, mk),
                            xsb[mk][:, 512 * (j4a + d):512 * (j4a + d + 1)],
                            start=(mk == 0), stop=(mk == MK - 1),
                        )
                for d in range(2):
                    j4 = j4a + d
                    # v^T slice + ones row -> vts bf16; 4 bf16 PE transposes
                    # into one merged [128, 260] PSUM tile; one DVE evac.
                    for h01 in range(2):
                        if len(vts_set) < 4:
                            vts = vtpool.tile([66, 512], BF16, tag="vT",
                                              name=f"vts{len(vts_set)}")
                            nc.gpsimd.memset(vts[64:66, :], 1.0)
                            vts_set.append(vts)
                        vts = vts_set[(2 * d + h01) % 4]
                        nc.scalar.add(
                            vts[0:64, :], pss[d][64 * h01:64 * (h01 + 1), :],
                            bslice("v", pr)[64 * h01:64 * (h01 + 1), :],
                        )
                        pstv = ps_vt.tile([128, 264], BF16, tag="vtps")
                        for c4 in range(4):
                            nc.tensor.transpose(
                                pstv[:, 66 * c4:66 * (c4 + 1)],
                                vts[:, 128 * c4:128 * (c4 + 1)], eye[:],
                            )
                        va = vapool.tile([128, 264], BF16, tag="va")
                        nc.vector.tensor_copy(
                            va.bitcast(mybir.dt.uint32),
                            pstv.bitcast(mybir.dt.uint32),
                        )
                        tiles[("va", pr, h01, j4)] = va
            for t, dest in (("q", qT), ("k", kT)):
                for j4a in range(0, PT, 2):
                    pss = [ps_qkv.tile([128, 512], F32, tag="qkvps",
                                       name=f"ps_{t}{pr}{j4a + d}")
                           for d in range(2)]
                    for mk in range(MK):
                        for d in range(2):
                            nc.tensor.matmul(
                                pss[d][:], wslice(t, pr, mk),
                                xsb[mk][:, 512 * (j4a + d):
                                         512 * (j4a + d + 1)],
                                start=(mk == 0), stop=(mk == MK - 1),
                            )
                    for d in range(2):
                        nc.scalar.add(
                            dest[:, 512 * (j4a + d):512 * (j4a + d + 1)],
                            pss[d][:], bslice(t, pr),
                        )

    # ---- deep-pipelined attention, j = PT-1 .. 0 ----
    with tc.tile_pool(name="ps_sc", bufs=2, space="PSUM") as ps_sc, \
         tc.tile_pool(name="ps_z", bufs=2, space="PSUM") as ps_z, \
         tc.tile_pool(name="ps_t", bufs=2, space="PSUM") as ps_t:
        for j in range(PT - 1, -1, -1):
            for pr in range(NPAIRS):
                attn(pr, j, ps_sc, ps_z, ps_t)
            proj(j, ps_t)


def _build():
    if "k" in _BUILT:
        return _BUILT["k"]
    from contextlib import ExitStack

    nc = bacc.Bacc("TRN2", target_bir_lowering=False, debug=False)
    aps = {
        "xT": nc.dram_tensor("xT", [M, P], BF16, kind="ExternalInput").ap(),
        "wall": nc.dram_tensor("wall", [128, 6 * 1024], BF16,
                               kind="ExternalInput").ap(),
        "wo": nc.dram_tensor("wo", [128, 2048], BF16,
                             kind="ExternalInput").ap(),
        "bb": nc.dram_tensor("bb", [128, 6], F32, kind="ExternalInput").ap(),
        "eye": nc.dram_tensor("eye", [66, 66], BF16,
                              kind="ExternalInput").ap(),
        "mask": nc.dram_tensor("mask", [128, 128], BF16,
                               kind="ExternalInput").ap(),
        "outp": nc.dram_tensor("outp", [P, M], F32, kind="ExternalOutput").ap(),
    }
    with tile.TileContext(nc) as tc:
        with ExitStack() as ctx:
            _emit(nc, tc, aps, ctx)
    nc.compile()
    _BUILT["k"] = nc
    return nc


def _host_inputs(x, kq, kk, kv, ko, bq, bk, bv):
    NP_MM = ml_dtypes.bfloat16
    xT = np.ascontiguousarray(x.transpose(0, 2, 1)).astype(NP_MM)  # [B, M, P]
    eye = np.eye(66, dtype=np.float32).astype(NP_MM)
    # keep iff pq < pk; block mask[r(pk), c(pq)] = 1 if c < r
    mask = np.tril(np.ones((128, 128), np.float32), k=-1).astype(NP_MM)
    in_maps = []
    for c in range(NCORES):
        b, k4 = divmod(c, 4)
        heads = [4 * k4 + i for i in range(HPC)]

        def pairw(kern, pr):
            # [128, MK*128]: m-chunk-major SBUF layout, contiguous in DRAM
            pairm = np.concatenate(
                [kern[heads[2 * pr]], kern[heads[2 * pr + 1]]], axis=1
            )  # [1024 m, 128 h']
            return pairm.reshape(MK, 128, 128).transpose(1, 0, 2).reshape(
                128, MK * 128)

        wall = np.concatenate(
            [pairw(kern, pr) for pr in range(NPAIRS)
             for kern in (kv, kq, kk)], axis=1
        ).astype(NP_MM)  # [128, 6*1024]

        bbias = np.stack(
            [np.concatenate([bias[heads[2 * pr]], bias[heads[2 * pr + 1]]])
             for pr in range(NPAIRS) for bias in (bv, bq, bk)], axis=1
        ).astype(np.float32)  # [128, 6]

        # [128 h', 1024 m] per pair -> [128, 2048] pair-major
        wo = np.concatenate(
            [np.concatenate([ko[heads[2 * pr]], ko[heads[2 * pr + 1]]],
                            axis=0) for pr in range(NPAIRS)], axis=1
        )

        in_maps.append({
            "xT": xT[b],
            "wall": wall,
            "wo": wo.astype(NP_MM),
            "bb": bbias,
            "eye": eye, "mask": mask,
        })
    return in_maps


def kernel(x, kernel_query, kernel_key, kernel_value, kernel_out,
           bias_query, bias_key, bias_value, bias_out, _trace=False):
    x = np.asarray(x, np.float32)
    kq = np.asarray(kernel_query, np.float32)
    kk = np.asarray(kernel_key, np.float32)
    kv = np.asarray(kernel_value, np.float32)
    ko = np.asarray(kernel_out, np.float32)
    bq = np.asarray(bias_query, np.float32)
    bk = np.asarray(bias_key, np.float32)
    bv = np.asarray(bias_value, np.float32)
    bo = np.asarray(bias_out, np.float32)

    nc = _build()
    in_maps = _host_inputs(x, kq, kk, kv, ko, bq, bk, bv)
    res = bass_utils.run_bass_kernel_spmd(
        nc, in_maps, core_ids=list(range(NCORES)), trace=_trace
    )
    out = np.zeros((B, P, M), np.float32)
    for c in range(NCORES):
        out[c // 4] += res.results[c]["outp"]
    out += bo[None, None, :]

    # patch fully-masked query row P-1: uniform attention = mean_k v
    for b in range(B):
        xbar = x[b].mean(axis=0, dtype=np.float64)  # [M]
        row = np.zeros(M, np.float64)
        for n in range(N):
            zrow = xbar @ kv[n].astype(np.float64) + bv[n].astype(np.float64)
            row += zrow @ ko[n].astype(np.float64)
        out[b, P - 1, :] = (row + bo.astype(np.float64)).astype(np.float32)

    if _trace:
        kernel._last_result = res
    return out


# revision 9
# speedup vs baseline: 1.0343x; 1.0343x over previous
"""Trainium2 Bass kernel for multi-head attention (B=2, P=2048, M=1024, N=16, H=64).

out = softmax(mask(x@Wq @ (x@Wk)^T / sqrt(H))) @ (x@Wv) @ Wo + biases,
with the module's strictly-upper-triangular keep mask (row P-1 fully masked).

Sharding: 8 cores = 2 batches x 4 head-groups. Core c handles batch c//4,
heads [4*(c%4), 4*(c%4)+4). Each core computes its heads' attention and the
partial output projection; the host sums partials across the 4 cores of each
batch.

Device algorithm (per core; bf16 matmuls, fp32 PSUM accumulation):
  - All projection weights arrive as ONE contiguous [128, 6*1024] DMA (host
    pre-packs the m-chunk-major layout); x^T arrives as 8 linear 512KB chunk
    DMAs alternating between the two HWDGE rings so the first projection
    matmul starts ~2us in.
  - q^T,k^T [h', p] via projections with x^T as the moving operand; QKV
    biases are folded into the PSUM evacuation (ACT Identity + per-partition
    bias AP), so no K=1 bias matmuls.
  - v^T slices + an appended ones row are PE-transposed (bf16) into merged
    [128, 4*65] PSUM tiles -> one DVE evacuation per 4 chunks. Column 64 of
    each 65-block is ones, so the z matmul also produces softmax
    denominators.
  - scores^T [pk, pq] with the triangular mask exploited by skipping
    fully-masked pk-chunks and narrowing partial ones. The two heads of a
    pair run CONCURRENTLY in disjoint PE row-groups (K=64 each), one ScalarE
    exp covers both heads; mask applied multiplicatively after exp.
  - z matmuls trail the scores/exp stream by a sliding window of DW slots.
  - Normalization WITHOUT transposes: the two denominator rows are
    reciprocal'd ([1,512] DVE ops from PSUM), broadcast down 128 partitions
    by one K=2 fp32 matmul against a constant selector, and multiplied into
    z^T by two DVE tensor_muls -> zp [128 (2 heads x 64), 512 pq] bf16,
    which is directly the lhsT for the output projection.
  - The fully-masked query row P-1 is patched analytically on the host:
    out[b,P-1] = sum_n (mean_p x[b] @ Wv[n] + bv[n]) @ Wo[n] + bias_out.
"""
import sys

import numpy as np

if "/opt/trn_rl_repo" not in sys.path:
    sys.path.insert(0, "/opt/trn_rl_repo")

import concourse.bacc as bacc
import concourse.tile as tile
from concourse import mybir
from concourse import bass_utils
import ml_dtypes

B, P, M, N, H = 2, 2048, 1024, 16, 64
NCORES = 8
HPC = 4          # heads per core
NPAIRS = 2       # head pairs per core
MK = M // 128    # 8 contraction chunks for projections
PT = P // 512    # 4 free-dim tiles of 512 over sequence
PC = P // 128    # 16 partition chunks over sequence
MT = M // 512    # 2 output m-tiles

F32 = mybir.dt.float32
BF16 = mybir.dt.bfloat16
EXP = mybir.ActivationFunctionType.Exp

_BUILT = {}


def _emit(nc, tc, aps, ctx):
    xT = aps["xT"]          # [1024, 2048]
    outp = aps["outp"]      # [2048, 1024]

    consts = ctx.enter_context(tc.tile_pool(name="consts", bufs=1))
    xpool = ctx.enter_context(tc.tile_pool(name="xpool", bufs=MK))
    qkpool = ctx.enter_context(tc.tile_pool(name="qkpool", bufs=4))
    vapool = ctx.enter_context(tc.tile_pool(name="vapool", bufs=18))
    zppool = ctx.enter_context(tc.tile_pool(name="zppool", bufs=9))
    expool = ctx.enter_context(tc.tile_pool(name="expool", bufs=9))
    rcpool = ctx.enter_context(tc.tile_pool(name="rcpool", bufs=2))
    bcpool = ctx.enter_context(tc.tile_pool(name="bcpool", bufs=2))
    opool = ctx.enter_context(tc.tile_pool(name="opool", bufs=4))
    vtpool = ctx.enter_context(tc.tile_pool(name="vtpool", bufs=4))

    # ---- constants + weights, packed for contiguous DMA ----
    bb = consts.tile([128, 6], F32, tag="bb")       # qkv biases, both pairs
    nc.sync.dma_start(bb[:], aps["bb"][:])
    eye = consts.tile([66, 66], BF16, tag="eye")
    nc.sync.dma_start(eye[:], aps["eye"][:])
    mask = consts.tile([128, 128], BF16, tag="mask")
    nc.sync.dma_start(mask[:], aps["mask"][:])
    ones1 = consts.tile([1, 128], F32, tag="ones1")
    nc.vector.memset(ones1[:], 1.0)

    # wall: [wv0 | wq0 | wk0 | wv1 | wq1 | wk1], each [128, MK*128]
    wall = consts.tile([128, 6 * 1024], BF16, tag="wall")
    nc.scalar.dma_start(wall[:, 0:1024], aps["wall"][:, 0:1024])
    nc.scalar.dma_start(wall[:, 1024:6144], aps["wall"][:, 1024:6144])
    wot = consts.tile([128, 2048], BF16, tag="wo")
    nc.scalar.dma_start(wot[:], aps["wo"][:])

    def wslice(t, pr, mk):
        base = 1024 * (3 * pr + {"v": 0, "q": 1, "k": 2}[t])
        return wall[:, base + 128 * mk:base + 128 * (mk + 1)]

    def bslice(t, pr):
        col = 3 * pr + {"v": 0, "q": 1, "k": 2}[t]
        return bb[:, col:col + 1]

    # x^T chunks [128 m, 2048 p], linear 512KB each, alternating rings
    xsb = []
    for k in range(MK):
        xt = xpool.tile([128, 2048], BF16, tag="x")
        eng = nc.sync if k % 2 == 0 else nc.scalar
        eng.dma_start(xt[:], xT[128 * k:128 * (k + 1), :])
        xsb.append(xt)

    tiles = {}
    qts, kts = {}, {}
    vts_set = []

    def finish_pair(pr, j, zpss, bc_pool):
        """Normalize both heads' z^T without transposes: reciprocal the
        denominator rows, broadcast down partitions via one K=2 matmul,
        multiply into z^T -> zp [128, 512] bf16 (lhsT of the out proj)."""
        if j == PT - 1:
            # fully-masked query row P-1: denom 0 -> 1 so the reciprocal
            # is finite (host patches the output row)
            nc.vector.memset(zpss[0][64:65, 511:512], 1.0)
            nc.vector.memset(zpss[1][64:65, 511:512], 1.0)
        dcs = [rcpool.tile([1, 512], F32, tag=f"dc{h01}",
                           name=f"dc{pr}_{j}_{h01}")
               for h01 in range(2)]
        rcs = [rcpool.tile([1, 512], F32, tag=f"rc{h01}",
                           name=f"rc{pr}_{j}_{h01}")
               for h01 in range(2)]
        for h01 in range(2):
            nc.vector.tensor_copy(dcs[h01][:], zpss[h01][64:65, :])
            nc.vector.reciprocal_approx_fast(rcs[h01][:], dcs[h01][:])
        bcps = bc_pool.tile([128, 512], F32, tag="tps", name=f"bc{pr}_{j}")
        for h01 in range(2):
            nc.tensor.matmul(bcps[64 * h01:64 * (h01 + 1), :],
                             ones1[:, 0:64], rcs[h01][:],
                             start=True, stop=True)
        bcs = bcpool.tile([128, 512], F32, tag="bcs")
        nc.scalar.copy(bcs[:], bcps[:])
        zp = zppool.tile([128, 512], BF16, tag="zp", name=f"zp{pr}_{j}")
        nc.vector.tensor_mul(zp[0:64, :], zpss[0][0:64, :], bcs[0:64, :])
        nc.vector.tensor_mul(zp[64:128, :], zpss[1][0:64, :], bcs[64:128, :])
        tiles[("zp", pr, j)] = zp

    def proj(j, ps_pool):
        for c4 in range(4):
            ck = 4 * j + c4
            for mt in range(MT):
                pp = ps_pool.tile([128, 512], F32, tag="tps", bufs=2,
                                  name=f"prps{ck}_{mt}")
                nc.tensor.matmul(
                    pp[:], tiles[("zp", 0, j)][:, 128 * c4:128 * (c4 + 1)],
                    wot[:, 1024 * 0 + 512 * mt:1024 * 0 + 512 * (mt + 1)],
                    start=True, stop=False,
                )
                nc.tensor.matmul(
                    pp[:], tiles[("zp", 1, j)][:, 128 * c4:128 * (c4 + 1)],
                    wot[:, 1024 * 1 + 512 * mt:1024 * 1 + 512 * (mt + 1)],
                    start=False, stop=True,
                )
                osb = opool.tile([128, 512], F32, tag="osb")
                if (c4 + mt) % 2 == 0:
                    nc.vector.tensor_copy(osb[:], pp[:])
                else:
                    nc.scalar.copy(osb[:], pp[:])
                nc.sync.dma_start(
                    outp[128 * ck:128 * (ck + 1), 512 * mt:512 * (mt + 1)],
                    osb[:],
                )

    def va_slice(pr, h01, i_):
        t, c4 = tiles[("va", pr, h01, i_ // 4)], i_ % 4
        return t[:, 66 * c4:66 * c4 + 65]

    def attn(pr, j, sc_pool, z_pool, bc_pool):
        """Row-packed attention: both heads' K=64 score matmuls run
        concurrently in disjoint PE row-groups into one [128,1024] PSUM
        tile; one batched exp covers both. z matmuls trail by DW slots."""
        qT, kT = qts[pr], kts[pr]
        ilist = list(range(PC - 1, 4 * j - 1, -1))
        nslot = len(ilist)
        DW = min(6, nslot - 1)
        zpss = [z_pool.tile([65, 512], F32, tag="zps", bufs=2,
                            name=f"zps{pr}_{h01}_{j}")
                for h01 in range(2)]
        descs = []
        for idx in range(nslot + DW):
            if idx < nslot:
                i_ = ilist[idx]
                tt = i_ - 4 * j
                w_ = min(512, 128 * (tt + 1))
                sp = sc_pool.tile([128, 1024], F32, tag="scps", bufs=2,
                                  name=f"sps{pr}_{j}_{i_}")
                halves = [sp[:, :w_], sp[:, 512:512 + w_]]
                for h01 in range(2):
                    rows = slice(64 * h01, 64 * (h01 + 1))
                    nc.tensor.matmul(
                        halves[h01],
                        kT[rows, 128 * i_:128 * (i_ + 1)],
                        qT[rows, 512 * j:512 * j + w_],
                        start=True, stop=True,
                    )
                ex = expool.tile([128, 1024], BF16, tag="ex")
                if w_ == 512:
                    nc.scalar.activation(ex[:], sp[:], EXP, scale=0.125)
                else:
                    nc.scalar.activation(ex[:, :w_], halves[0], EXP,
                                         scale=0.125)
                    nc.scalar.activation(ex[:, 512:512 + w_], halves[1],
                                         EXP, scale=0.125)
                if tt < 4:
                    for h01 in range(2):
                        off = 512 * h01
                        nc.vector.tensor_mul(
                            ex[:, off + 128 * tt:off + w_],
                            ex[:, off + 128 * tt:off + w_], mask[:]
                        )
                descs.append((ex, i_, w_))
            zi = idx - DW
            if 0 <= zi < nslot:
                ex, i_, w_ = descs[zi]
                for h01 in range(2):
                    nc.tensor.matmul(
                        zpss[h01][:, :w_], va_slice(pr, h01, i_),
                        ex[:, 512 * h01:512 * h01 + w_],
                        start=(zi == 0), stop=(zi == nslot - 1),
                    )
        finish_pair(pr, j, zpss, bc_pool)

    # ---- QKV projections ----
    with tc.tile_pool(name="ps_qkv", bufs=3, space="PSUM") as ps_qkv, \
         tc.tile_pool(name="ps_vt", bufs=1, space="PSUM") as ps_vt:
        for pr in range(NPAIRS):
            qT = qkpool.tile([128, 2048], BF16, tag="qT", name=f"qT{pr}")
            kT = qkpool.tile([128, 2048], BF16, tag="kT", name=f"kT{pr}")
            qts[pr], kts[pr] = qT, kT
            # v first: va tiles feed the z matmuls of the j=3 attention
            for j4a in range(0, PT, 2):
                pss = [ps_qkv.tile([128, 512], F32, tag="qkvps",
                                   name=f"ps_v{pr}{j4a + d}")
                       for d in range(2)]
                for mk in range(MK):
                    for d in range(2):
                        nc.tensor.matmul(
                            pss[d][:], wslice("v", pr, mk),
                            xsb[mk][:, 512 * (j4a + d):512 * (j4a + d + 1)],
                            start=(mk == 0), stop=(mk == MK - 1),
                        )
                for d in range(2):
                    j4 = j4a + d
                    # v^T slice + ones row -> vts bf16; 4 bf16 PE transposes
                    # into one merged [128, 260] PSUM tile; one DVE evac.
                    for h01 in range(2):
                        if len(vts_set) < 4:
                            vts = vtpool.tile([66, 512], BF16, tag="vT",
                                              name=f"vts{len(vts_set)}")
                            nc.gpsimd.memset(vts[64:66, :], 1.0)
                            vts_set.append(vts)
                        vts = vts_set[(2 * d + h01) % 4]
                        nc.scalar.add(
                            vts[0:64, :], pss[d][64 * h01:64 * (h01 + 1), :],
                            bslice("v", pr)[64 * h01:64 * (h01 + 1), :],
                        )
                        pstv = ps_vt.tile([128, 264], BF16, tag="vtps")
                        for c4 in range(4):
                            nc.tensor.transpose(
                                pstv[:, 66 * c4:66 * (c4 + 1)],
                                vts[:, 128 * c4:128 * (c4 + 1)], eye[:],
                            )
                        va = vapool.tile([128, 264], BF16, tag="va")
                        nc.vector.tensor_copy(
                            va.bitcast(mybir.dt.uint32),
                            pstv.bitcast(mybir.dt.uint32),
                        )
                        tiles[("va", pr, h01, j4)] = va
            for t, dest in (("q", qT), ("k", kT)):
                for j4a in range(0, PT, 2):
                    pss = [ps_qkv.tile([128, 512], F32, tag="qkvps",
                                       name=f"ps_{t}{pr}{j4a + d}")
                           for d in range(2)]
                    for mk in range(MK):
                        for d in range(2):
                            nc.tensor.matmul(
                                pss[d][:], wslice(t, pr, mk),
                                xsb[mk][:, 512 * (j4a + d):
                                         512 * (j4a + d + 1)],
                                start=(mk == 0), stop=(mk == MK - 1),
                            )
                    for d in range(2):
                        nc.scalar.add(
                            dest[:, 512 * (j4a + d):512 * (j4a + d + 1)],
                            pss[d][:], bslice(t, pr),
                        )

    # ---- deep-pipelined attention, j = PT-1 .. 0 ----
    with tc.tile_pool(name="ps_sc", bufs=2, space="PSUM") as ps_sc, \
         tc.tile_pool(name="ps_z", bufs=2, space="PSUM") as ps_z, \
         tc.tile_pool(name="ps_t", bufs=2, space="PSUM") as ps_t:
        for j in range(PT - 1, -1, -1):
            for pr in range(NPAIRS):
                attn(pr, j, ps_sc, ps_z, ps_t)
            proj(j, ps_t)


def _build():
    if "k" in _BUILT:
        return _BUILT["k"]
    from contextlib import ExitStack

    nc = bacc.Bacc("TRN2", target_bir_lowering=False, debug=False)
    aps = {
        "xT": nc.dram_tensor("xT", [M, P], BF16, kind="ExternalInput").ap(),
        "wall": nc.dram_tensor("wall", [128, 6 * 1024], BF16,
                               kind="ExternalInput").ap(),
        "wo": nc.dram_tensor("wo", [128, 2048], BF16,
                             kind="ExternalInput").ap(),
        "bb": nc.dram_tensor("bb", [128, 6], F32, kind="ExternalInput").ap(),
        "eye": nc.dram_tensor("eye", [66, 66], BF16,
                              kind="ExternalInput").ap(),
        "mask": nc.dram_tensor("mask", [128, 128], BF16,
                               kind="ExternalInput").ap(),
        "outp": nc.dram_tensor("outp", [P, M], F32, kind="ExternalOutput").ap(),
    }
    with tile.TileContext(nc) as tc:
        with ExitStack() as ctx:
            _emit(nc, tc, aps, ctx)
    nc.compile()
    _BUILT["k"] = nc
    return nc


def _host_inputs(x, kq, kk, kv, ko, bq, bk, bv):
    NP_MM = ml_dtypes.bfloat16
    xT = np.ascontiguousarray(x.transpose(0, 2, 1)).astype(NP_MM)  # [B, M, P]
    eye = np.eye(66, dtype=np.float32).astype(NP_MM)
    # keep iff pq < pk; block mask[r(pk), c(pq)] = 1 if c < r
    mask = np.tril(np.ones((128, 128), np.float32), k=-1).astype(NP_MM)
    in_maps = []
    for c in range(NCORES):
        b, k4 = divmod(c, 4)
        heads = [4 * k4 + i for i in range(HPC)]

        def pairw(kern, pr):
            # [128, MK*128]: m-chunk-major SBUF layout, contiguous in DRAM
            pairm = np.concatenate(
                [kern[heads[2 * pr]], kern[heads[2 * pr + 1]]], axis=1
            )  # [1024 m, 128 h']
            return pairm.reshape(MK, 128, 128).transpose(1, 0, 2).reshape(
                128, MK * 128)

        wall = np.concatenate(
            [pairw(kern, pr) for pr in range(NPAIRS)
             for kern in (kv, kq, kk)], axis=1
        ).astype(NP_MM)  # [128, 6*1024]

        bbias = np.stack(
            [np.concatenate([bias[heads[2 * pr]], bias[heads[2 * pr + 1]]])
             for pr in range(NPAIRS) for bias in (bv, bq, bk)], axis=1
        ).astype(np.float32)  # [128, 6]

        # [128 h', 1024 m] per pair -> [128, 2048] pair-major
        wo = np.concatenate(
            [np.concatenate([ko[heads[2 * pr]], ko[heads[2 * pr + 1]]],
                            axis=0) for pr in range(NPAIRS)], axis=1
        )

        in_maps.append({
            "xT": xT[b],
            "wall": wall,
            "wo": wo.astype(NP_MM),
            "bb": bbias,
            "eye": eye, "mask": mask,
        })
    return in_maps


def kernel(x, kernel_query, kernel_key, kernel_value, kernel_out,
           bias_query, bias_key, bias_value, bias_out, _trace=False):
    x = np.asarray(x, np.float32)
    kq = np.asarray(kernel_query, np.float32)
    kk = np.asarray(kernel_key, np.float32)
    kv = np.asarray(kernel_value, np.float32)
    ko = np.asarray(kernel_out, np.float32)
    bq = np.asarray(bias_query, np.float32)
    bk = np.asarray(bias_key, np.float32)
    bv = np.asarray(bias_value, np.float32)
    bo = np.asarray(bias_out, np.float32)

    nc = _build()
    in_maps = _host_inputs(x, kq, kk, kv, ko, bq, bk, bv)
    res = bass_utils.run_bass_kernel_spmd(
        nc, in_maps, core_ids=list(range(NCORES)), trace=_trace
    )
    out = np.zeros((B, P, M), np.float32)
    for c in range(NCORES):
        out[c // 4] += res.results[c]["outp"]
    out += bo[None, None, :]

    # patch fully-masked query row P-1: uniform attention = mean_k v
    for b in range(B):
        xbar = x[b].mean(axis=0, dtype=np.float64)  # [M]
        row = np.zeros(M, np.float64)
        for n in range(N):
            zrow = xbar @ kv[n].astype(np.float64) + bv[n].astype(np.float64)
            row += zrow @ ko[n].astype(np.float64)
        out[b, P - 1, :] = (row + bo.astype(np.float64)).astype(np.float32)

    if _trace:
        kernel._last_result = res
    return out


# revision 10
# speedup vs baseline: 1.0526x; 1.0177x over previous
"""Trainium2 Bass kernel for multi-head attention (B=2, P=2048, M=1024, N=16, H=64).

out = softmax(mask(x@Wq @ (x@Wk)^T / sqrt(H))) @ (x@Wv) @ Wo + biases,
with the module's strictly-upper-triangular keep mask (row P-1 fully masked).

Sharding: 8 cores = 2 batches x 4 head-groups. Core c handles batch c//4,
heads [4*(c%4), 4*(c%4)+4). Each core computes its heads' attention and the
partial output projection; the host sums partials across the 4 cores of each
batch.

Device algorithm (per core; bf16 matmuls, fp32 PSUM accumulation):
  - All projection weights arrive as ONE contiguous [128, 6*1024] DMA (host
    pre-packs the m-chunk-major layout); x^T arrives as 8 linear 512KB chunk
    DMAs alternating between the two HWDGE rings so the first projection
    matmul starts ~2us in.
  - q^T,k^T [h', p] via projections with x^T as the moving operand; QKV
    biases are folded into the PSUM evacuation (ACT Identity + per-partition
    bias AP), so no K=1 bias matmuls.
  - v^T slices + an appended ones row are PE-transposed (bf16) into merged
    [128, 4*65] PSUM tiles -> one DVE evacuation per 4 chunks. Column 64 of
    each 65-block is ones, so the z matmul also produces softmax
    denominators.
  - scores^T [pk, pq] with the triangular mask exploited by skipping
    fully-masked pk-chunks and narrowing partial ones. The two heads of a
    pair run CONCURRENTLY in disjoint PE row-groups (K=64 each), one ScalarE
    exp covers both heads; mask applied multiplicatively after exp.
  - z matmuls trail the scores/exp stream by a sliding window of DW slots.
  - Normalization WITHOUT transposes: the two denominator rows are
    reciprocal'd ([1,512] DVE ops from PSUM), broadcast down 128 partitions
    by one K=2 fp32 matmul against a constant selector, and multiplied into
    z^T by two DVE tensor_muls -> zp [128 (2 heads x 64), 512 pq] bf16,
    which is directly the lhsT for the output projection.
  - The fully-masked query row P-1 is patched analytically on the host:
    out[b,P-1] = sum_n (mean_p x[b] @ Wv[n] + bv[n]) @ Wo[n] + bias_out.
"""
import sys

import numpy as np

if "/opt/trn_rl_repo" not in sys.path:
    sys.path.insert(0, "/opt/trn_rl_repo")

import concourse.bacc as bacc
import concourse.tile as tile
from concourse import mybir
from concourse import bass_utils
import ml_dtypes

B, P, M, N, H = 2, 2048, 1024, 16, 64
NCORES = 8
HPC = 4          # heads per core
NPAIRS = 2       # head pairs per core
MK = M // 128    # 8 contraction chunks for projections
PT = P // 512    # 4 free-dim tiles of 512 over sequence
PC = P // 128    # 16 partition chunks over sequence
MT = M // 512    # 2 output m-tiles

F32 = mybir.dt.float32
BF16 = mybir.dt.bfloat16
EXP = mybir.ActivationFunctionType.Exp

_BUILT = {}


def _emit(nc, tc, aps, ctx):
    xT = aps["xT"]          # [1024, 2048]
    outp = aps["outp"]      # [2048, 1024]

    consts = ctx.enter_context(tc.tile_pool(name="consts", bufs=1))
    xpool = ctx.enter_context(tc.tile_pool(name="xpool", bufs=MK))
    qkpool = ctx.enter_context(tc.tile_pool(name="qkpool", bufs=4))
    vapool = ctx.enter_context(tc.tile_pool(name="vapool", bufs=18))
    zppool = ctx.enter_context(tc.tile_pool(name="zppool", bufs=9))
    expool = ctx.enter_context(tc.tile_pool(name="expool", bufs=9))
    rcpool = ctx.enter_context(tc.tile_pool(name="rcpool", bufs=2))
    bcpool = ctx.enter_context(tc.tile_pool(name="bcpool", bufs=2))
    opool = ctx.enter_context(tc.tile_pool(name="opool", bufs=4))
    vtpool = ctx.enter_context(tc.tile_pool(name="vtpool", bufs=4))

    # ---- constants + weights, packed for contiguous DMA; issue order
    # puts first-needed tensors at the head of both HWDGE rings ----
    ones1 = consts.tile([1, 128], F32, tag="ones1")
    nc.vector.memset(ones1[:], 1.0)
    # wall: [wv0 | wq0 | wk0 | wv1 | wq1 | wk1], each [128, MK*128]
    wall = consts.tile([128, 6 * 1024], BF16, tag="wall")
    nc.scalar.dma_start(wall[:, 0:3072], aps["wall"][:, 0:3072])
    bb = consts.tile([128, 6], F32, tag="bb")       # qkv biases, both pairs
    nc.sync.dma_start(bb[:], aps["bb"][:])

    def wslice(t, pr, mk):
        base = 1024 * (3 * pr + {"v": 0, "q": 1, "k": 2}[t])
        return wall[:, base + 128 * mk:base + 128 * (mk + 1)]

    def bslice(t, pr):
        col = 3 * pr + {"v": 0, "q": 1, "k": 2}[t]
        return bb[:, col:col + 1]

    # x^T chunks [128 m, 2048 p], linear 512KB each, alternating rings
    xsb = []
    for k in range(MK):
        xt = xpool.tile([128, 2048], BF16, tag="x")
        eng = nc.scalar if k % 2 == 0 else nc.sync
        eng.dma_start(xt[:], xT[128 * k:128 * (k + 1), :])
        xsb.append(xt)
        if k == 0:
            nc.scalar.dma_start(wall[:, 3072:6144], aps["wall"][:, 3072:6144])
        if k == 1:
            eye = consts.tile([66, 66], BF16, tag="eye")
            nc.sync.dma_start(eye[:], aps["eye"][:])
            mask = consts.tile([128, 128], BF16, tag="mask")
            nc.sync.dma_start(mask[:], aps["mask"][:])
    wot = consts.tile([128, 2048], BF16, tag="wo")
    nc.scalar.dma_start(wot[:], aps["wo"][:])

    tiles = {}
    qts, kts = {}, {}
    vts_set = []

    def finish_pair(pr, j, zpss, bc_pool):
        """Normalize both heads' z^T without transposes: reciprocal the
        denominator rows, broadcast down partitions via one K=2 matmul,
        multiply into z^T -> zp [128, 512] bf16 (lhsT of the out proj)."""
        if j == PT - 1:
            # fully-masked query row P-1: denom 0 -> 1 so the reciprocal
            # is finite (host patches the output row)
            nc.vector.memset(zpss[0][64:65, 511:512], 1.0)
            nc.vector.memset(zpss[1][64:65, 511:512], 1.0)
        dcs = [rcpool.tile([1, 512], F32, tag=f"dc{h01}",
                           name=f"dc{pr}_{j}_{h01}")
               for h01 in range(2)]
        rcs = [rcpool.tile([1, 512], F32, tag=f"rc{h01}",
                           name=f"rc{pr}_{j}_{h01}")
               for h01 in range(2)]
        for h01 in range(2):
            nc.vector.tensor_copy(dcs[h01][:], zpss[h01][64:65, :])
            nc.vector.reciprocal_approx_fast(rcs[h01][:], dcs[h01][:])
        bcps = bc_pool.tile([128, 512], F32, tag="tps", name=f"bc{pr}_{j}")
        for h01 in range(2):
            nc.tensor.matmul(bcps[64 * h01:64 * (h01 + 1), :],
                             ones1[:, 0:64], rcs[h01][:],
                             start=True, stop=True)
        bcs = bcpool.tile([128, 512], F32, tag="bcs")
        nc.scalar.copy(bcs[:], bcps[:])
        zp = zppool.tile([128, 512], BF16, tag="zp", name=f"zp{pr}_{j}")
        nc.vector.tensor_mul(zp[0:64, :], zpss[0][0:64, :], bcs[0:64, :])
        nc.vector.tensor_mul(zp[64:128, :], zpss[1][0:64, :], bcs[64:128, :])
        tiles[("zp", pr, j)] = zp

    def proj(j, ps_pool):
        for c4 in range(4):
            ck = 4 * j + c4
            for mt in range(MT):
                pp = ps_pool.tile([128, 512], F32, tag="tps", bufs=2,
                                  name=f"prps{ck}_{mt}")
                nc.tensor.matmul(
                    pp[:], tiles[("zp", 0, j)][:, 128 * c4:128 * (c4 + 1)],
                    wot[:, 1024 * 0 + 512 * mt:1024 * 0 + 512 * (mt + 1)],
                    start=True, stop=False,
                )
                nc.tensor.matmul(
                    pp[:], tiles[("zp", 1, j)][:, 128 * c4:128 * (c4 + 1)],
                    wot[:, 1024 * 1 + 512 * mt:1024 * 1 + 512 * (mt + 1)],
                    start=False, stop=True,
                )
                osb = opool.tile([128, 512], F32, tag="osb")
                nc.vector.tensor_copy(osb[:], pp[:])
                nc.sync.dma_start(
                    outp[128 * ck:128 * (ck + 1), 512 * mt:512 * (mt + 1)],
                    osb[:],
                )

    def va_slice(pr, h01, i_):
        t, c4 = tiles[("va", pr, h01, i_ // 4)], i_ % 4
        return t[:, 66 * c4:66 * c4 + 65]

    def attn(pr, j, sc_pool, z_pool, bc_pool):
        """Row-packed attention: both heads' K=64 score matmuls run
        concurrently in disjoint PE row-groups into one [128,1024] PSUM
        tile; one batched exp covers both. z matmuls trail by DW slots."""
        qT, kT = qts[pr], kts[pr]
        ilist = list(range(PC - 1, 4 * j - 1, -1))
        nslot = len(ilist)
        DW = min(6, nslot - 1)
        zpss = [z_pool.tile([65, 512], F32, tag="zps", bufs=2,
                            name=f"zps{pr}_{h01}_{j}")
                for h01 in range(2)]
        descs = []
        for idx in range(nslot + DW):
            if idx < nslot:
                i_ = ilist[idx]
                tt = i_ - 4 * j
                w_ = min(512, 128 * (tt + 1))
                sp = sc_pool.tile([128, 1024], F32, tag="scps", bufs=2,
                                  name=f"sps{pr}_{j}_{i_}")
                halves = [sp[:, :w_], sp[:, 512:512 + w_]]
                for h01 in range(2):
                    rows = slice(64 * h01, 64 * (h01 + 1))
                    nc.tensor.matmul(
                        halves[h01],
                        kT[rows, 128 * i_:128 * (i_ + 1)],
                        qT[rows, 512 * j:512 * j + w_],
                        start=True, stop=True,
                    )
                ex = expool.tile([128, 1024], BF16, tag="ex")
                if w_ == 512:
                    nc.scalar.activation(ex[:], sp[:], EXP, scale=0.125)
                else:
                    nc.scalar.activation(ex[:, :w_], halves[0], EXP,
                                         scale=0.125)
                    nc.scalar.activation(ex[:, 512:512 + w_], halves[1],
                                         EXP, scale=0.125)
                if tt < 4:
                    for h01 in range(2):
                        off = 512 * h01
                        nc.vector.tensor_mul(
                            ex[:, off + 128 * tt:off + w_],
                            ex[:, off + 128 * tt:off + w_], mask[:]
                        )
                descs.append((ex, i_, w_))
            zi = idx - DW
            if 0 <= zi < nslot:
                ex, i_, w_ = descs[zi]
                for h01 in range(2):
                    nc.tensor.matmul(
                        zpss[h01][:, :w_], va_slice(pr, h01, i_),
                        ex[:, 512 * h01:512 * h01 + w_],
                        start=(zi == 0), stop=(zi == nslot - 1),
                    )
        finish_pair(pr, j, zpss, bc_pool)

    # ---- QKV projections: v/q/k interleaved per m-chunk so the PE has
    # 6 matmuls of work per arriving x chunk during the DMA ramp ----
    with tc.tile_pool(name="ps_qkv", bufs=6, space="PSUM") as ps_qkv, \
         tc.tile_pool(name="ps_vt", bufs=2, space="PSUM") as ps_vt:
        for pr in range(NPAIRS):
            qT = qkpool.tile([128, 2048], BF16, tag="qT", name=f"qT{pr}")
            kT = qkpool.tile([128, 2048], BF16, tag="kT", name=f"kT{pr}")
            qts[pr], kts[pr] = qT, kT
            for j4a in range(0, PT, 2):
                pst = {}
                for t in ("v", "q", "k"):
                    for d in range(2):
                        pp = ps_qkv.tile([128, 512], F32, tag="qkvps",
                                         name=f"ps_{t}{pr}{j4a + d}")
                        pst[(t, d)] = pp
                for mk in range(MK):
                    for t in ("v", "q", "k"):
                        for d in range(2):
                            nc.tensor.matmul(
                                pst[(t, d)][:], wslice(t, pr, mk),
                                xsb[mk][:, 512 * (j4a + d):
                                         512 * (j4a + d + 1)],
                                start=(mk == 0), stop=(mk == MK - 1),
                            )
                for t, dest in (("q", qT), ("k", kT)):
                    for d in range(2):
                        nc.scalar.add(
                            dest[:, 512 * (j4a + d):512 * (j4a + d + 1)],
                            pst[(t, d)][:], bslice(t, pr),
                        )
                for d in range(2):
                    j4 = j4a + d
                    # v^T slice + ones row -> vts bf16; 4 bf16 PE transposes
                    # into one merged [128, 264] PSUM tile; one DVE evac.
                    for h01 in range(2):
                        if len(vts_set) < 4:
                            vts = vtpool.tile([66, 512], BF16, tag="vT",
                                              name=f"vts{len(vts_set)}")
                            nc.gpsimd.memset(vts[64:66, :], 1.0)
                            vts_set.append(vts)
                        vts = vts_set[(2 * d + h01) % 4]
                        nc.vector.tensor_scalar_add(
                            vts[0:64, :],
                            pst[("v", d)][64 * h01:64 * (h01 + 1), :],
                            bslice("v", pr)[64 * h01:64 * (h01 + 1), :],
                        )
                        pstv = ps_vt.tile([128, 264], BF16, tag="vtps")
                        for c4 in range(4):
                            nc.tensor.transpose(
                                pstv[:, 66 * c4:66 * (c4 + 1)],
                                vts[:, 128 * c4:128 * (c4 + 1)], eye[:],
                            )
                        va = vapool.tile([128, 264], BF16, tag="va")
                        nc.vector.tensor_copy(
                            va.bitcast(mybir.dt.uint32),
                            pstv.bitcast(mybir.dt.uint32),
                        )
                        tiles[("va", pr, h01, j4)] = va

    # ---- deep-pipelined attention, j = PT-1 .. 0 ----
    with tc.tile_pool(name="ps_sc", bufs=2, space="PSUM") as ps_sc, \
         tc.tile_pool(name="ps_z", bufs=2, space="PSUM") as ps_z, \
         tc.tile_pool(name="ps_t", bufs=2, space="PSUM") as ps_t:
        for j in range(PT - 1, -1, -1):
            for pr in range(NPAIRS):
                attn(pr, j, ps_sc, ps_z, ps_t)
            proj(j, ps_t)


def _build():
    if "k" in _BUILT:
        return _BUILT["k"]
    from contextlib import ExitStack

    nc = bacc.Bacc("TRN2", target_bir_lowering=False, debug=False)
    aps = {
        "xT": nc.dram_tensor("xT", [M, P], BF16, kind="ExternalInput").ap(),
        "wall": nc.dram_tensor("wall", [128, 6 * 1024], BF16,
                               kind="ExternalInput").ap(),
        "wo": nc.dram_tensor("wo", [128, 2048], BF16,
                             kind="ExternalInput").ap(),
        "bb": nc.dram_tensor("bb", [128, 6], F32, kind="ExternalInput").ap(),
        "eye": nc.dram_tensor("eye", [66, 66], BF16,
                              kind="ExternalInput").ap(),
        "mask": nc.dram_tensor("mask", [128, 128], BF16,
                               kind="ExternalInput").ap(),
        "outp": nc.dram_tensor("outp", [P, M], F32, kind="ExternalOutput").ap(),
    }
    with tile.TileContext(nc) as tc:
        with ExitStack() as ctx:
            _emit(nc, tc, aps, ctx)
    nc.compile()
    _BUILT["k"] = nc
    return nc


def _host_inputs(x, kq, kk, kv, ko, bq, bk, bv):
    NP_MM = ml_dtypes.bfloat16
    xT = np.ascontiguousarray(x.transpose(0, 2, 1)).astype(NP_MM)  # [B, M, P]
    eye = np.eye(66, dtype=np.float32).astype(NP_MM)
    # keep iff pq < pk; block mask[r(pk), c(pq)] = 1 if c < r
    mask = np.tril(np.ones((128, 128), np.float32), k=-1).astype(NP_MM)
    in_maps = []
    for c in range(NCORES):
        b, k4 = divmod(c, 4)
        heads = [4 * k4 + i for i in range(HPC)]

        def pairw(kern, pr):
            # [128, MK*128]: m-chunk-major SBUF layout, contiguous in DRAM
            pairm = np.concatenate(
                [kern[heads[2 * pr]], kern[heads[2 * pr + 1]]], axis=1
            )  # [1024 m, 128 h']
            return pairm.reshape(MK, 128, 128).transpose(1, 0, 2).reshape(
                128, MK * 128)

        wall = np.concatenate(
            [pairw(kern, pr) for pr in range(NPAIRS)
             for kern in (kv, kq, kk)], axis=1
        ).astype(NP_MM)  # [128, 6*1024]

        bbias = np.stack(
            [np.concatenate([bias[heads[2 * pr]], bias[heads[2 * pr + 1]]])
             for pr in range(NPAIRS) for bias in (bv, bq, bk)], axis=1
        ).astype(np.float32)  # [128, 6]

        # [128 h', 1024 m] per pair -> [128, 2048] pair-major
        wo = np.concatenate(
            [np.concatenate([ko[heads[2 * pr]], ko[heads[2 * pr + 1]]],
                            axis=0) for pr in range(NPAIRS)], axis=1
        )

        in_maps.append({
            "xT": xT[b],
            "wall": wall,
            "wo": wo.astype(NP_MM),
            "bb": bbias,
            "eye": eye, "mask": mask,
        })
    return in_maps


def kernel(x, kernel_query, kernel_key, kernel_value, kernel_out,
           bias_query, bias_key, bias_value, bias_out, _trace=False):
    x = np.asarray(x, np.float32)
    kq = np.asarray(kernel_query, np.float32)
    kk = np.asarray(kernel_key, np.float32)
    kv = np.asarray(kernel_value, np.float32)
    ko = np.asarray(kernel_out, np.float32)
    bq = np.asarray(bias_query, np.float32)
    bk = np.asarray(bias_key, np.float32)
    bv = np.asarray(bias_value, np.float32)
    bo = np.asarray(bias_out, np.float32)

    nc = _build()
    in_maps = _host_inputs(x, kq, kk, kv, ko, bq, bk, bv)
    res = bass_utils.run_bass_kernel_spmd(
        nc, in_maps, core_ids=list(range(NCORES)), trace=_trace
    )
    out = np.zeros((B, P, M), np.float32)
    for c in range(NCORES):
        out[c // 4] += res.results[c]["outp"]
    out += bo[None, None, :]

    # patch fully-masked query row P-1: uniform attention = mean_k v
    for b in range(B):
        xbar = x[b].mean(axis=0, dtype=np.float64)  # [M]
        row = np.zeros(M, np.float64)
        for n in range(N):
            zrow = xbar @ kv[n].astype(np.float64) + bv[n].astype(np.float64)
            row += zrow @ ko[n].astype(np.float64)
        out[b, P - 1, :] = (row + bo.astype(np.float64)).astype(np.float32)

    if _trace:
        kernel._last_result = res
    return out


# revision 12
# speedup vs baseline: 1.2585x; 1.1955x over previous
"""Trainium2 Bass kernel for multi-head attention (B=2, P=2048, M=1024, N=16, H=64).

out = softmax(mask(x@Wq @ (x@Wk)^T / sqrt(H))) @ (x@Wv) @ Wo + biases,
with the module's strictly-upper-triangular keep mask (row P-1 fully masked).

Sharding: 8 cores = 2 batches x 4 head-groups. Core c handles batch c//4,
heads [4*(c%4), 4*(c%4)+4). Each core computes its heads' attention and the
partial output projection; the host sums partials across the 4 cores of each
batch.

Device algorithm (per core; bf16 matmuls, fp32 PSUM accumulation):
  - All projection weights arrive as ONE contiguous [128, 6*1024] DMA (host
    pre-packs the m-chunk-major layout); x^T arrives as 8 linear 512KB chunk
    DMAs alternating between the two HWDGE rings so the first projection
    matmul starts ~2us in.
  - q^T,k^T [h', p] via projections with x^T as the moving operand; QKV
    biases are folded into the PSUM evacuation (ACT Identity + per-partition
    bias AP), so no K=1 bias matmuls.
  - v^T slices + an appended ones row are PE-transposed (bf16) into merged
    [128, 4*65] PSUM tiles -> one DVE evacuation per 4 chunks. Column 64 of
    each 65-block is ones, so the z matmul also produces softmax
    denominators.
  - scores^T [pk, pq] with the triangular mask exploited by skipping
    fully-masked pk-chunks and narrowing partial ones. The two heads of a
    pair run CONCURRENTLY in disjoint PE row-groups (K=64 each), one ScalarE
    exp covers both heads; mask applied multiplicatively after exp.
  - z matmuls trail the scores/exp stream by a sliding window of DW slots.
  - Normalization WITHOUT transposes: the two denominator rows are
    reciprocal'd ([1,512] DVE ops from PSUM), broadcast down 128 partitions
    by one K=2 fp32 matmul against a constant selector, and multiplied into
    z^T by two DVE tensor_muls -> zp [128 (2 heads x 64), 512 pq] bf16,
    which is directly the lhsT for the output projection.
  - The fully-masked query row P-1 is patched analytically on the host:
    out[b,P-1] = sum_n (mean_p x[b] @ Wv[n] + bv[n]) @ Wo[n] + bias_out.
"""
import sys

import numpy as np

if "/opt/trn_rl_repo" not in sys.path:
    sys.path.insert(0, "/opt/trn_rl_repo")

import concourse.bacc as bacc
import concourse.tile as tile
from concourse import mybir
from concourse import bass_utils
import ml_dtypes

B, P, M, N, H = 2, 2048, 1024, 16, 64
NCORES = 8
HPC = 4          # heads per core
NPAIRS = 2       # head pairs per core
MK = M // 128    # 8 contraction chunks for projections
PT = P // 512    # 4 free-dim tiles of 512 over sequence
PC = P // 128    # 16 partition chunks over sequence
MT = M // 512    # 2 output m-tiles

F32 = mybir.dt.float32
BF16 = mybir.dt.bfloat16
EXP = mybir.ActivationFunctionType.Exp

_BUILT = {}


def _emit(nc, tc, aps, ctx):
    xT = aps["xT"]          # [1024, 2048]
    outp = aps["outp"]      # [2048, 1024]

    consts = ctx.enter_context(tc.tile_pool(name="consts", bufs=1))
    xpool = ctx.enter_context(tc.tile_pool(name="xpool", bufs=MK))
    qkpool = ctx.enter_context(tc.tile_pool(name="qkpool", bufs=4))
    vapool = ctx.enter_context(tc.tile_pool(name="vapool", bufs=18))
    zppool = ctx.enter_context(tc.tile_pool(name="zppool", bufs=9))
    expool = ctx.enter_context(tc.tile_pool(name="expool", bufs=9))
    rcpool = ctx.enter_context(tc.tile_pool(name="rcpool", bufs=2))
    bcpool = ctx.enter_context(tc.tile_pool(name="bcpool", bufs=2))
    opool = ctx.enter_context(tc.tile_pool(name="opool", bufs=4))
    vtpool = ctx.enter_context(tc.tile_pool(name="vtpool", bufs=4))

    # ---- constants + weights, packed for contiguous DMA; issue order
    # puts first-needed tensors at the head of both HWDGE rings ----
    ones1 = consts.tile([1, 128], F32, tag="ones1")
    nc.vector.memset(ones1[:], 1.0)
    # wall: per pair, mk-interleaved [wv_mk | wq_mk | wk_mk] blocks of 384
    # cols, so the first matmul's weights are in the first 192KB DMA
    wall = consts.tile([128, 6 * 1024], BF16, tag="wall")
    nc.scalar.dma_start(wall[:, 0:768], aps["wall"][:, 0:768])

    def wslice(t, pr, mk):
        base = 3072 * pr + 384 * mk + 128 * {"v": 0, "q": 1, "k": 2}[t]
        return wall[:, base:base + 128]

    def bslice(t, pr):
        col = 3 * pr + {"v": 0, "q": 1, "k": 2}[t]
        return bb[:, col:col + 1]

    # x^T chunks [128 m, 2048 p], linear 512KB each, alternating rings
    xsb = []
    for k in range(MK):
        xt = xpool.tile([128, 2048], BF16, tag="x")
        eng = nc.scalar if k % 2 == 0 else nc.sync
        eng.dma_start(xt[:], xT[128 * k:128 * (k + 1), :])
        xsb.append(xt)
        if k == 0:
            nc.scalar.dma_start(wall[:, 768:3072], aps["wall"][:, 768:3072])
        if k == 1:
            bb = consts.tile([128, 6], F32, tag="bb")   # qkv biases
            nc.sync.dma_start(bb[:], aps["bb"][:])
            eye = consts.tile([66, 66], BF16, tag="eye")
            nc.sync.dma_start(eye[:], aps["eye"][:])
            mask = consts.tile([128, 128], BF16, tag="mask")
            nc.sync.dma_start(mask[:], aps["mask"][:])
        if k == 2:
            nc.scalar.dma_start(wall[:, 3072:6144], aps["wall"][:, 3072:6144])
    wot = consts.tile([128, 2048], BF16, tag="wo")
    nc.scalar.dma_start(wot[:], aps["wo"][:])

    tiles = {}
    qts, kts = {}, {}
    vts_set = []

    def finish_pair(pr, j, zpss, bc_pool):
        """Normalize both heads' z^T without transposes: reciprocal the
        denominator rows, broadcast down partitions via one K=2 matmul,
        multiply into z^T -> zp [128, 512] bf16 (lhsT of the out proj)."""
        if j == PT - 1:
            # fully-masked query row P-1: denom 0 -> 1 so the reciprocal
            # is finite (host patches the output row)
            nc.vector.memset(zpss[0][64:65, 511:512], 1.0)
            nc.vector.memset(zpss[1][64:65, 511:512], 1.0)
        dcs = [rcpool.tile([1, 512], F32, tag=f"dc{h01}",
                           name=f"dc{pr}_{j}_{h01}")
               for h01 in range(2)]
        rcs = [rcpool.tile([1, 512], F32, tag=f"rc{h01}",
                           name=f"rc{pr}_{j}_{h01}")
               for h01 in range(2)]
        for h01 in range(2):
            nc.scalar.copy(dcs[h01][:], zpss[h01][64:65, :])
            nc.vector.reciprocal_approx_fast(rcs[h01][:], dcs[h01][:])
        bcps = bc_pool.tile([128, 512], F32, tag="tps", name=f"bc{pr}_{j}")
        for h01 in range(2):
            nc.tensor.matmul(bcps[64 * h01:64 * (h01 + 1), :],
                             ones1[:, 0:64], rcs[h01][:],
                             start=True, stop=True)
        bcs = bcpool.tile([128, 512], F32, tag="bcs")
        nc.vector.tensor_copy(bcs[:], bcps[:])
        zp = zppool.tile([128, 512], BF16, tag="zp", name=f"zp{pr}_{j}")
        nc.vector.tensor_mul(zp[0:64, :], zpss[0][0:64, :], bcs[0:64, :])
        nc.vector.tensor_mul(zp[64:128, :], zpss[1][0:64, :], bcs[64:128, :])
        tiles[("zp", pr, j)] = zp

    def proj(j, ps_pool):
        for c4 in range(4):
            ck = 4 * j + c4
            for mt in range(MT):
                pp = ps_pool.tile([128, 512], F32, tag="tps", bufs=2,
                                  name=f"prps{ck}_{mt}")
                nc.tensor.matmul(
                    pp[:], tiles[("zp", 0, j)][:, 128 * c4:128 * (c4 + 1)],
                    wot[:, 1024 * 0 + 512 * mt:1024 * 0 + 512 * (mt + 1)],
                    start=True, stop=False,
                )
                nc.tensor.matmul(
                    pp[:], tiles[("zp", 1, j)][:, 128 * c4:128 * (c4 + 1)],
                    wot[:, 1024 * 1 + 512 * mt:1024 * 1 + 512 * (mt + 1)],
                    start=False, stop=True,
                )
                osb = opool.tile([128, 512], F32, tag="osb")
                nc.vector.tensor_copy(osb[:], pp[:])
                nc.sync.dma_start(
                    outp[128 * ck:128 * (ck + 1), 512 * mt:512 * (mt + 1)],
                    osb[:],
                )

    def va_slice(pr, h01, i_):
        t, c4 = tiles[("va", pr, h01, i_ // 4)], i_ % 4
        return t[:, 66 * c4:66 * c4 + 65]

    def attn(pr, j, sc_pool, z_pool, bc_pool):
        """Row-packed attention: both heads' K=64 score matmuls run
        concurrently in disjoint PE row-groups into one [128,1024] PSUM
        tile; one batched exp covers both. z matmuls trail by DW slots."""
        qT, kT = qts[pr], kts[pr]
        ilist = list(range(PC - 1, 4 * j - 1, -1))
        nslot = len(ilist)
        DW = min(6, nslot - 1)
        zpss = [z_pool.tile([65, 512], F32, tag="zps", bufs=2,
                            name=f"zps{pr}_{h01}_{j}")
                for h01 in range(2)]
        descs = []
        for idx in range(nslot + DW):
            if idx < nslot:
                i_ = ilist[idx]
                tt = i_ - 4 * j
                w_ = min(512, 128 * (tt + 1))
                sp = sc_pool.tile([128, 1024], F32, tag="scps", bufs=2,
                                  name=f"sps{pr}_{j}_{i_}")
                halves = [sp[:, :w_], sp[:, 512:512 + w_]]
                for h01 in range(2):
                    rows = slice(64 * h01, 64 * (h01 + 1))
                    nc.tensor.matmul(
                        halves[h01],
                        kT[rows, 128 * i_:128 * (i_ + 1)],
                        qT[rows, 512 * j:512 * j + w_],
                        start=True, stop=True,
                    )
                ex = expool.tile([128, 1024], BF16, tag="ex")
                if w_ == 512:
                    nc.scalar.activation(ex[:], sp[:], EXP, scale=0.125)
                else:
                    nc.scalar.activation(ex[:, :w_], halves[0], EXP,
                                         scale=0.125)
                    nc.scalar.activation(ex[:, 512:512 + w_], halves[1],
                                         EXP, scale=0.125)
                if tt < 4:
                    for h01 in range(2):
                        off = 512 * h01
                        nc.vector.tensor_mul(
                            ex[:, off + 128 * tt:off + w_],
                            ex[:, off + 128 * tt:off + w_], mask[:]
                        )
                descs.append((ex, i_, w_))
            zi = idx - DW
            if 0 <= zi < nslot:
                ex, i_, w_ = descs[zi]
                for h01 in range(2):
                    nc.tensor.matmul(
                        zpss[h01][:, :w_], va_slice(pr, h01, i_),
                        ex[:, 512 * h01:512 * h01 + w_],
                        start=(zi == 0), stop=(zi == nslot - 1),
                    )
        finish_pair(pr, j, zpss, bc_pool)

    # ---- QKV projections: v/q/k interleaved per m-chunk so the PE has
    # 6 matmuls of work per arriving x chunk during the DMA ramp ----
    with tc.tile_pool(name="ps_qkv", bufs=6, space="PSUM") as ps_qkv, \
         tc.tile_pool(name="ps_vt", bufs=2, space="PSUM") as ps_vt:
        for pr in range(NPAIRS):
            qT = qkpool.tile([128, 2048], BF16, tag="qT", name=f"qT{pr}")
            kT = qkpool.tile([128, 2048], BF16, tag="kT", name=f"kT{pr}")
            qts[pr], kts[pr] = qT, kT
            for j4a in range(0, PT, 2):
                pst = {}
                for t in ("v", "q", "k"):
                    for d in range(2):
                        pp = ps_qkv.tile([128, 512], F32, tag="qkvps",
                                         name=f"ps_{t}{pr}{j4a + d}")
                        pst[(t, d)] = pp
                for mk in range(MK):
                    for t in ("v", "q", "k"):
                        for d in range(2):
                            nc.tensor.matmul(
                                pst[(t, d)][:], wslice(t, pr, mk),
                                xsb[mk][:, 512 * (j4a + d):
                                         512 * (j4a + d + 1)],
                                start=(mk == 0), stop=(mk == MK - 1),
                            )
                for t, dest in (("q", qT), ("k", kT)):
                    for d in range(2):
                        nc.scalar.add(
                            dest[:, 512 * (j4a + d):512 * (j4a + d + 1)],
                            pst[(t, d)][:], bslice(t, pr),
                        )
                for d in range(2):
                    j4 = j4a + d
                    # v^T slice + ones row -> vts bf16; 4 bf16 PE transposes
                    # into one merged [128, 264] PSUM tile; one DVE evac.
                    for h01 in range(2):
                        if len(vts_set) < 4:
                            vts = vtpool.tile([66, 512], BF16, tag="vT",
                                              name=f"vts{len(vts_set)}")
                            nc.gpsimd.memset(vts[64:66, :], 1.0)
                            vts_set.append(vts)
                        vts = vts_set[(2 * d + h01) % 4]
                        nc.vector.tensor_scalar_add(
                            vts[0:64, :],
                            pst[("v", d)][64 * h01:64 * (h01 + 1), :],
                            bslice("v", pr)[64 * h01:64 * (h01 + 1), :],
                        )
                        pstv = ps_vt.tile([128, 264], BF16, tag="vtps")
                        for c4 in range(4):
                            nc.tensor.transpose(
                                pstv[:, 66 * c4:66 * (c4 + 1)],
                                vts[:, 128 * c4:128 * (c4 + 1)], eye[:],
                            )
                        va = vapool.tile([128, 264], BF16, tag="va")
                        nc.vector.tensor_copy(
                            va.bitcast(mybir.dt.uint32),
                            pstv.bitcast(mybir.dt.uint32),
                        )
                        tiles[("va", pr, h01, j4)] = va

    # ---- deep-pipelined attention, j = PT-1 .. 0 ----
    with tc.tile_pool(name="ps_sc", bufs=2, space="PSUM") as ps_sc, \
         tc.tile_pool(name="ps_z", bufs=2, space="PSUM") as ps_z, \
         tc.tile_pool(name="ps_t", bufs=2, space="PSUM") as ps_t:
        for j in range(PT - 1, -1, -1):
            for pr in range(NPAIRS):
                attn(pr, j, ps_sc, ps_z, ps_t)
            proj(j, ps_t)


def _build():
    if "k" in _BUILT:
        return _BUILT["k"]
    from contextlib import ExitStack

    nc = bacc.Bacc("TRN2", target_bir_lowering=False, debug=False)
    aps = {
        "xT": nc.dram_tensor("xT", [M, P], BF16, kind="ExternalInput").ap(),
        "wall": nc.dram_tensor("wall", [128, 6 * 1024], BF16,
                               kind="ExternalInput").ap(),
        "wo": nc.dram_tensor("wo", [128, 2048], BF16,
                             kind="ExternalInput").ap(),
        "bb": nc.dram_tensor("bb", [128, 6], F32, kind="ExternalInput").ap(),
        "eye": nc.dram_tensor("eye", [66, 66], BF16,
                              kind="ExternalInput").ap(),
        "mask": nc.dram_tensor("mask", [128, 128], BF16,
                               kind="ExternalInput").ap(),
        "outp": nc.dram_tensor("outp", [P, M], F32, kind="ExternalOutput").ap(),
    }
    with tile.TileContext(nc) as tc:
        with ExitStack() as ctx:
            _emit(nc, tc, aps, ctx)
    nc.compile()
    _BUILT["k"] = nc
    return nc


def _host_inputs(x, kq, kk, kv, ko, bq, bk, bv):
    NP_MM = ml_dtypes.bfloat16
    xT = np.ascontiguousarray(x.transpose(0, 2, 1)).astype(NP_MM)  # [B, M, P]
    eye = np.eye(66, dtype=np.float32).astype(NP_MM)
    # keep iff pq < pk; block mask[r(pk), c(pq)] = 1 if c < r
    mask = np.tril(np.ones((128, 128), np.float32), k=-1).astype(NP_MM)
    in_maps = []
    for c in range(NCORES):
        b, k4 = divmod(c, 4)
        heads = [4 * k4 + i for i in range(HPC)]

        def pairw(kern, pr):
            # [128 p, MK, 128 f] m-chunk-major
            pairm = np.concatenate(
                [kern[heads[2 * pr]], kern[heads[2 * pr + 1]]], axis=1
            )  # [1024 m, 128 h']
            return pairm.reshape(MK, 128, 128).transpose(1, 0, 2)

        # per pair: mk-interleaved [wv_mk | wq_mk | wk_mk] 384-col blocks
        wall = np.concatenate(
            [np.concatenate([pairw(kern, pr) for kern in (kv, kq, kk)],
                            axis=2).reshape(128, 3 * MK * 128)
             for pr in range(NPAIRS)], axis=1
        ).astype(NP_MM)  # [128, 6*1024]

        bbias = np.stack(
            [np.concatenate([bias[heads[2 * pr]], bias[heads[2 * pr + 1]]])
             for pr in range(NPAIRS) for bias in (bv, bq, bk)], axis=1
        ).astype(np.float32)  # [128, 6]

        # [128 h', 1024 m] per pair -> [128, 2048] pair-major
        wo = np.concatenate(
            [np.concatenate([ko[heads[2 * pr]], ko[heads[2 * pr + 1]]],
                            axis=0) for pr in range(NPAIRS)], axis=1
        )

        in_maps.append({
            "xT": xT[b],
            "wall": wall,
            "wo": wo.astype(NP_MM),
            "bb": bbias,
            "eye": eye, "mask": mask,
        })
    return in_maps


def kernel(x, kernel_query, kernel_key, kernel_value, kernel_out,
           bias_query, bias_key, bias_value, bias_out, _trace=False):
    x = np.asarray(x, np.float32)
    kq = np.asarray(kernel_query, np.float32)
    kk = np.asarray(kernel_key, np.float32)
    kv = np.asarray(kernel_value, np.float32)
    ko = np.asarray(kernel_out, np.float32)
    bq = np.asarray(bias_query, np.float32)
    bk = np.asarray(bias_key, np.float32)
    bv = np.asarray(bias_value, np.float32)
    bo = np.asarray(bias_out, np.float32)

    nc = _build()
    in_maps = _host_inputs(x, kq, kk, kv, ko, bq, bk, bv)
    res = bass_utils.run_bass_kernel_spmd(
        nc, in_maps, core_ids=list(range(NCORES)), trace=_trace
    )
    out = np.zeros((B, P, M), np.float32)
    for c in range(NCORES):
        out[c // 4] += res.results[c]["outp"]
    out += bo[None, None, :]

    # patch fully-masked query row P-1: uniform attention = mean_k v
    for b in range(B):
        xbar = x[b].mean(axis=0, dtype=np.float64)  # [M]
        row = np.zeros(M, np.float64)
        for n in range(N):
            zrow = xbar @ kv[n].astype(np.float64) + bv[n].astype(np.float64)
            row += zrow @ ko[n].astype(np.float64)
        out[b, P - 1, :] = (row + bo.astype(np.float64)).astype(np.float32)

    if _trace:
        kernel._last_result = res
    return out


# revision 13
# speedup vs baseline: 1.2743x; 1.0126x over previous
"""Trainium2 Bass kernel for multi-head attention (B=2, P=2048, M=1024, N=16, H=64).

out = softmax(mask(x@Wq @ (x@Wk)^T / sqrt(H))) @ (x@Wv) @ Wo + biases,
with the module's strictly-upper-triangular keep mask (row P-1 fully masked).

Sharding: 8 cores = 2 batches x 4 head-groups. Core c handles batch c//4,
heads [4*(c%4), 4*(c%4)+4). Each core computes its heads' attention and the
partial output projection; the host sums partials across the 4 cores of each
batch.

Device algorithm (per core; bf16 matmuls, fp32 PSUM accumulation):
  - All projection weights arrive as ONE contiguous [128, 6*1024] DMA (host
    pre-packs the m-chunk-major layout); x^T arrives as 8 linear 512KB chunk
    DMAs alternating between the two HWDGE rings so the first projection
    matmul starts ~2us in.
  - q^T,k^T [h', p] via projections with x^T as the moving operand; QKV
    biases are folded into the PSUM evacuation (ACT Identity + per-partition
    bias AP), so no K=1 bias matmuls.
  - v^T slices + an appended ones row are PE-transposed (bf16) into merged
    [128, 4*65] PSUM tiles -> one DVE evacuation per 4 chunks. Column 64 of
    each 65-block is ones, so the z matmul also produces softmax
    denominators.
  - scores^T [pk, pq] with the triangular mask exploited by skipping
    fully-masked pk-chunks and narrowing partial ones. The two heads of a
    pair run CONCURRENTLY in disjoint PE row-groups (K=64 each), one ScalarE
    exp covers both heads; mask applied multiplicatively after exp.
  - z matmuls trail the scores/exp stream by a sliding window of DW slots.
  - Normalization WITHOUT transposes: the two denominator rows are
    reciprocal'd ([1,512] DVE ops from PSUM), broadcast down 128 partitions
    by one K=2 fp32 matmul against a constant selector, and multiplied into
    z^T by two DVE tensor_muls -> zp [128 (2 heads x 64), 512 pq] bf16,
    which is directly the lhsT for the output projection.
  - The fully-masked query row P-1 is patched analytically on the host:
    out[b,P-1] = sum_n (mean_p x[b] @ Wv[n] + bv[n]) @ Wo[n] + bias_out.
"""
import sys

import numpy as np

if "/opt/trn_rl_repo" not in sys.path:
    sys.path.insert(0, "/opt/trn_rl_repo")

import concourse.bacc as bacc
import concourse.tile as tile
from concourse import mybir
from concourse import bass_utils
import ml_dtypes

B, P, M, N, H = 2, 2048, 1024, 16, 64
NCORES = 8
HPC = 4          # heads per core
NPAIRS = 2       # head pairs per core
MK = M // 128    # 8 contraction chunks for projections
PT = P // 512    # 4 free-dim tiles of 512 over sequence
PC = P // 128    # 16 partition chunks over sequence
MT = M // 512    # 2 output m-tiles

F32 = mybir.dt.float32
BF16 = mybir.dt.bfloat16
EXP = mybir.ActivationFunctionType.Exp

_BUILT = {}


def _emit(nc, tc, aps, ctx):
    xT = aps["xT"]          # [1024, 2048]
    outp = aps["outp"]      # [2048, 1024]

    consts = ctx.enter_context(tc.tile_pool(name="consts", bufs=1))
    xpool = ctx.enter_context(tc.tile_pool(name="xpool", bufs=MK))
    qkpool = ctx.enter_context(tc.tile_pool(name="qkpool", bufs=4))
    vapool = ctx.enter_context(tc.tile_pool(name="vapool", bufs=18))
    zppool = ctx.enter_context(tc.tile_pool(name="zppool", bufs=9))
    expool = ctx.enter_context(tc.tile_pool(name="expool", bufs=9))
    rcpool = ctx.enter_context(tc.tile_pool(name="rcpool", bufs=2))
    bcpool = ctx.enter_context(tc.tile_pool(name="bcpool", bufs=2))
    opool = ctx.enter_context(tc.tile_pool(name="opool", bufs=4))
    vtpool = ctx.enter_context(tc.tile_pool(name="vtpool", bufs=4))

    # ---- constants + weights, packed for contiguous DMA; issue order
    # puts first-needed tensors at the head of both HWDGE rings ----
    ones1 = consts.tile([1, 128], F32, tag="ones1")
    nc.vector.memset(ones1[:], 1.0)
    # wall: per pair, mk-interleaved [wv_mk | wq_mk | wk_mk] blocks of 384
    # cols, so the first matmul's weights are in the first 192KB DMA
    wall = consts.tile([128, 6 * 1024], BF16, tag="wall")
    nc.scalar.dma_start(wall[:, 0:768], aps["wall"][:, 0:768])

    def wslice(t, pr, mk):
        base = 3072 * pr + 384 * mk + 128 * {"v": 0, "q": 1, "k": 2}[t]
        return wall[:, base:base + 128]

    def bslice(t, pr):
        col = 3 * pr + {"v": 0, "q": 1, "k": 2}[t]
        return bb[:, col:col + 1]

    # x^T chunks [128 m, 2048 p], linear 512KB each, alternating rings
    xsb = []
    for k in range(MK):
        xt = xpool.tile([128, 2048], BF16, tag="x")
        eng = nc.scalar if k in (0, 4, 6) else nc.sync
        eng.dma_start(xt[:], xT[128 * k:128 * (k + 1), :])
        xsb.append(xt)
        if k == 0:
            nc.scalar.dma_start(wall[:, 768:3072], aps["wall"][:, 768:3072])
        if k == 1:
            bb = consts.tile([128, 6], F32, tag="bb")   # qkv biases
            nc.sync.dma_start(bb[:], aps["bb"][:])
            eye = consts.tile([66, 66], BF16, tag="eye")
            nc.sync.dma_start(eye[:], aps["eye"][:])
            mask = consts.tile([128, 128], BF16, tag="mask")
            nc.sync.dma_start(mask[:], aps["mask"][:])
        if k == 2:
            nc.scalar.dma_start(wall[:, 3072:6144], aps["wall"][:, 3072:6144])
    wot = consts.tile([128, 2048], BF16, tag="wo")
    nc.scalar.dma_start(wot[:], aps["wo"][:])

    tiles = {}
    qts, kts = {}, {}
    vts_set = []

    pending = [None]

    def flush_pending():
        if pending[0] is not None:
            fn, pending[0] = pending[0], None
            fn()

    def finish_pair(pr, j, zpss, bc_pool):
        """Normalize both heads' z^T without transposes: reciprocal the
        denominator rows (emitted now), then broadcast down partitions via
        two K=1 matmuls + multiply into z^T (deferred into the next slot's
        PE stream so the PE queue never waits on the reciprocals)."""
        if j == PT - 1:
            # fully-masked query row P-1: denom 0 -> 1 so the reciprocal
            # is finite (host patches the output row)
            nc.vector.memset(zpss[0][64:65, 511:512], 1.0)
            nc.vector.memset(zpss[1][64:65, 511:512], 1.0)
        dcs = [rcpool.tile([1, 512], F32, tag=f"dc{h01}",
                           name=f"dc{pr}_{j}_{h01}")
               for h01 in range(2)]
        rcs = [rcpool.tile([1, 512], F32, tag=f"rc{h01}",
                           name=f"rc{pr}_{j}_{h01}")
               for h01 in range(2)]
        for h01 in range(2):
            nc.scalar.copy(dcs[h01][:], zpss[h01][64:65, :])
            nc.vector.reciprocal_approx_fast(rcs[h01][:], dcs[h01][:])
        zp = zppool.tile([128, 512], BF16, tag="zp", name=f"zp{pr}_{j}")
        tiles[("zp", pr, j)] = zp

        def part_b():
            bcps = bc_pool.tile([128, 512], F32, tag="tps",
                                name=f"bc{pr}_{j}")
            for h01 in range(2):
                nc.tensor.matmul(bcps[64 * h01:64 * (h01 + 1), :],
                                 ones1[:, 0:64], rcs[h01][:],
                                 start=True, stop=True)
            bcs = bcpool.tile([128, 512], F32, tag="bcs")
            nc.vector.tensor_copy(bcs[:], bcps[:])
            nc.vector.tensor_mul(zp[0:64, :], zpss[0][0:64, :], bcs[0:64, :])
            nc.vector.tensor_mul(zp[64:128, :], zpss[1][0:64, :],
                                 bcs[64:128, :])

        flush_pending()
        pending[0] = part_b

    def proj(j, ps_pool):
        for c4 in range(4):
            ck = 4 * j + c4
            for mt in range(MT):
                pp = ps_pool.tile([128, 512], F32, tag="tps", bufs=2,
                                  name=f"prps{ck}_{mt}")
                nc.tensor.matmul(
                    pp[:], tiles[("zp", 0, j)][:, 128 * c4:128 * (c4 + 1)],
                    wot[:, 1024 * 0 + 512 * mt:1024 * 0 + 512 * (mt + 1)],
                    start=True, stop=False,
                )
                nc.tensor.matmul(
                    pp[:], tiles[("zp", 1, j)][:, 128 * c4:128 * (c4 + 1)],
                    wot[:, 1024 * 1 + 512 * mt:1024 * 1 + 512 * (mt + 1)],
                    start=False, stop=True,
                )
                osb = opool.tile([128, 512], F32, tag="osb")
                nc.vector.tensor_copy(osb[:], pp[:])
                nc.sync.dma_start(
                    outp[128 * ck:128 * (ck + 1), 512 * mt:512 * (mt + 1)],
                    osb[:],
                )

    def va_slice(pr, h01, i_):
        t, c4 = tiles[("va", pr, h01, i_ // 4)], i_ % 4
        return t[:, 66 * c4:66 * c4 + 65]

    def attn(pr, j, sc_pool, z_pool, bc_pool):
        """Row-packed attention: both heads' K=64 score matmuls run
        concurrently in disjoint PE row-groups into one [128,1024] PSUM
        tile; one batched exp covers both. z matmuls trail by DW slots."""
        qT, kT = qts[pr], kts[pr]
        ilist = list(range(PC - 1, 4 * j - 1, -1))
        nslot = len(ilist)
        DW = min(6, nslot - 1)
        zpss = [z_pool.tile([65, 512], F32, tag="zps", bufs=2,
                            name=f"zps{pr}_{h01}_{j}")
                for h01 in range(2)]
        descs = []
        for idx in range(nslot + DW):
            if idx < nslot:
                i_ = ilist[idx]
                tt = i_ - 4 * j
                w_ = min(512, 128 * (tt + 1))
                sp = sc_pool.tile([128, 1024], F32, tag="scps", bufs=2,
                                  name=f"sps{pr}_{j}_{i_}")
                halves = [sp[:, :w_], sp[:, 512:512 + w_]]
                for h01 in range(2):
                    rows = slice(64 * h01, 64 * (h01 + 1))
                    nc.tensor.matmul(
                        halves[h01],
                        kT[rows, 128 * i_:128 * (i_ + 1)],
                        qT[rows, 512 * j:512 * j + w_],
                        start=True, stop=True,
                    )
                if idx == 1:
                    flush_pending()
                ex = expool.tile([128, 1024], BF16, tag="ex")
                if w_ == 512:
                    nc.scalar.activation(ex[:], sp[:], EXP, scale=0.125)
                else:
                    nc.scalar.activation(ex[:, :w_], halves[0], EXP,
                                         scale=0.125)
                    nc.scalar.activation(ex[:, 512:512 + w_], halves[1],
                                         EXP, scale=0.125)
                if tt < 4:
                    for h01 in range(2):
                        off = 512 * h01
                        nc.vector.tensor_mul(
                            ex[:, off + 128 * tt:off + w_],
                            ex[:, off + 128 * tt:off + w_], mask[:]
                        )
                descs.append((ex, i_, w_))
            zi = idx - DW
            if 0 <= zi < nslot:
                ex, i_, w_ = descs[zi]
                for h01 in range(2):
                    nc.tensor.matmul(
                        zpss[h01][:, :w_], va_slice(pr, h01, i_),
                        ex[:, 512 * h01:512 * h01 + w_],
                        start=(zi == 0), stop=(zi == nslot - 1),
                    )
        finish_pair(pr, j, zpss, bc_pool)

    # ---- QKV projections: v/q/k interleaved per m-chunk so the PE has
    # 6 matmuls of work per arriving x chunk during the DMA ramp ----
    with tc.tile_pool(name="ps_qkv", bufs=6, space="PSUM") as ps_qkv, \
         tc.tile_pool(name="ps_vt", bufs=2, space="PSUM") as ps_vt:
        for pr in range(NPAIRS):
            qT = qkpool.tile([128, 2048], BF16, tag="qT", name=f"qT{pr}")
            kT = qkpool.tile([128, 2048], BF16, tag="kT", name=f"kT{pr}")
            qts[pr], kts[pr] = qT, kT
            for j4a in range(0, PT, 2):
                pst = {}
                for t in ("v", "q", "k"):
                    for d in range(2):
                        pp = ps_qkv.tile([128, 512], F32, tag="qkvps",
                                         name=f"ps_{t}{pr}{j4a + d}")
                        pst[(t, d)] = pp
                for mk in range(MK):
                    for t in ("v", "q", "k"):
                        for d in range(2):
                            nc.tensor.matmul(
                                pst[(t, d)][:], wslice(t, pr, mk),
                                xsb[mk][:, 512 * (j4a + d):
                                         512 * (j4a + d + 1)],
                                start=(mk == 0), stop=(mk == MK - 1),
                            )
                for t, dest in (("q", qT), ("k", kT)):
                    for d in range(2):
                        nc.scalar.add(
                            dest[:, 512 * (j4a + d):512 * (j4a + d + 1)],
                            pst[(t, d)][:], bslice(t, pr),
                        )
                for d in range(2):
                    j4 = j4a + d
                    # v^T slice + ones row -> vts bf16; 4 bf16 PE transposes
                    # into one merged [128, 264] PSUM tile; one DVE evac.
                    for h01 in range(2):
                        if len(vts_set) < 4:
                            vts = vtpool.tile([66, 512], BF16, tag="vT",
                                              name=f"vts{len(vts_set)}")
                            nc.gpsimd.memset(vts[64:66, :], 1.0)
                            vts_set.append(vts)
                        vts = vts_set[(2 * d + h01) % 4]
                        nc.vector.tensor_scalar_add(
                            vts[0:64, :],
                            pst[("v", d)][64 * h01:64 * (h01 + 1), :],
                            bslice("v", pr)[64 * h01:64 * (h01 + 1), :],
                        )
                        pstv = ps_vt.tile([128, 264], BF16, tag="vtps")
                        for c4 in range(4):
                            nc.tensor.transpose(
                                pstv[:, 66 * c4:66 * (c4 + 1)],
                                vts[:, 128 * c4:128 * (c4 + 1)], eye[:],
                            )
                        va = vapool.tile([128, 264], BF16, tag="va")
                        nc.vector.tensor_copy(
                            va.bitcast(mybir.dt.uint32),
                            pstv.bitcast(mybir.dt.uint32),
                        )
                        tiles[("va", pr, h01, j4)] = va

    # ---- deep-pipelined attention, j = PT-1 .. 0 ----
    with tc.tile_pool(name="ps_sc", bufs=2, space="PSUM") as ps_sc, \
         tc.tile_pool(name="ps_z", bufs=2, space="PSUM") as ps_z, \
         tc.tile_pool(name="ps_t", bufs=2, space="PSUM") as ps_t:
        prev_j = None
        for j in range(PT - 1, -1, -1):
            for pr in range(NPAIRS):
                attn(pr, j, ps_sc, ps_z, ps_t)
                if pr == 1 and prev_j is not None:
                    proj(prev_j, ps_t)
            prev_j = j
        flush_pending()
        proj(0, ps_t)


def _build():
    if "k" in _BUILT:
        return _BUILT["k"]
    from contextlib import ExitStack

    nc = bacc.Bacc("TRN2", target_bir_lowering=False, debug=False)
    aps = {
        "xT": nc.dram_tensor("xT", [M, P], BF16, kind="ExternalInput").ap(),
        "wall": nc.dram_tensor("wall", [128, 6 * 1024], BF16,
                               kind="ExternalInput").ap(),
        "wo": nc.dram_tensor("wo", [128, 2048], BF16,
                             kind="ExternalInput").ap(),
        "bb": nc.dram_tensor("bb", [128, 6], F32, kind="ExternalInput").ap(),
        "eye": nc.dram_tensor("eye", [66, 66], BF16,
                              kind="ExternalInput").ap(),
        "mask": nc.dram_tensor("mask", [128, 128], BF16,
                               kind="ExternalInput").ap(),
        "outp": nc.dram_tensor("outp", [P, M], F32, kind="ExternalOutput").ap(),
    }
    with tile.TileContext(nc) as tc:
        with ExitStack() as ctx:
            _emit(nc, tc, aps, ctx)
    nc.compile()
    _BUILT["k"] = nc
    return nc


def _host_inputs(x, kq, kk, kv, ko, bq, bk, bv):
    NP_MM = ml_dtypes.bfloat16
    xT = np.ascontiguousarray(x.transpose(0, 2, 1)).astype(NP_MM)  # [B, M, P]
    eye = np.eye(66, dtype=np.float32).astype(NP_MM)
    # keep iff pq < pk; block mask[r(pk), c(pq)] = 1 if c < r
    mask = np.tril(np.ones((128, 128), np.float32), k=-1).astype(NP_MM)
    in_maps = []
    for c in range(NCORES):
        b, k4 = divmod(c, 4)
        heads = [4 * k4 + i for i in range(HPC)]

        def pairw(kern, pr):
            # [128 p, MK, 128 f] m-chunk-major
            pairm = np.concatenate(
                [kern[heads[2 * pr]], kern[heads[2 * pr + 1]]], axis=1
            )  # [1024 m, 128 h']
            return pairm.reshape(MK, 128, 128).transpose(1, 0, 2)

        # per pair: mk-interleaved [wv_mk | wq_mk | wk_mk] 384-col blocks
        wall = np.concatenate(
            [np.concatenate([pairw(kern, pr) for kern in (kv, kq, kk)],
                            axis=2).reshape(128, 3 * MK * 128)
             for pr in range(NPAIRS)], axis=1
        ).astype(NP_MM)  # [128, 6*1024]

        bbias = np.stack(
            [np.concatenate([bias[heads[2 * pr]], bias[heads[2 * pr + 1]]])
             for pr in range(NPAIRS) for bias in (bv, bq, bk)], axis=1
        ).astype(np.float32)  # [128, 6]

        # [128 h', 1024 m] per pair -> [128, 2048] pair-major
        wo = np.concatenate(
            [np.concatenate([ko[heads[2 * pr]], ko[heads[2 * pr + 1]]],
                            axis=0) for pr in range(NPAIRS)], axis=1
        )

        in_maps.append({
            "xT": xT[b],
            "wall": wall,
            "wo": wo.astype(NP_MM),
            "bb": bbias,
            "eye": eye, "mask": mask,
        })
    return in_maps


def kernel(x, kernel_query, kernel_key, kernel_value, kernel_out,
           bias_query, bias_key, bias_value, bias_out, _trace=False):
    x = np.asarray(x, np.float32)
    kq = np.asarray(kernel_query, np.float32)
    kk = np.asarray(kernel_key, np.float32)
    kv = np.asarray(kernel_value, np.float32)
    ko = np.asarray(kernel_out, np.float32)
    bq = np.asarray(bias_query, np.float32)
    bk = np.asarray(bias_key, np.float32)
    bv = np.asarray(bias_value, np.float32)
    bo = np.asarray(bias_out, np.float32)

    nc = _build()
    in_maps = _host_inputs(x, kq, kk, kv, ko, bq, bk, bv)
    res = bass_utils.run_bass_kernel_spmd(
        nc, in_maps, core_ids=list(range(NCORES)), trace=_trace
    )
    out = np.zeros((B, P, M), np.float32)
    for c in range(NCORES):
        out[c // 4] += res.results[c]["outp"]
    out += bo[None, None, :]

    # patch fully-masked query row P-1: uniform attention = mean_k v
    for b in range(B):
        xbar = x[b].mean(axis=0, dtype=np.float64)  # [M]
        row = np.zeros(M, np.float64)
        for n in range(N):
            zrow = xbar @ kv[n].astype(np.float64) + bv[n].astype(np.float64)
            row += zrow @ ko[n].astype(np.float64)
        out[b, P - 1, :] = (row + bo.astype(np.float64)).astype(np.float32)

    if _trace:
        kernel._last_result = res
    return out
